# revision 12
# baseline (speedup 1.0000x reference)
"""BiDiTreeLSTM Trainium2 kernel (v3: bf16 datapath, tuned pipelines).

Full-input contract: kernel(**inputs) takes the unsharded numpy inputs of
reference.setup_inputs() and returns the full [64, 512] output.

Strategy: data-parallel over trees (8 trees per NeuronCore, 8 cores).
Per-core layout is feature-major: every node-state tensor lives in SBUF as
[128 partitions, 2 feature-chunk column halves] ("g-major"), where within a
half the columns are level-major blocks, tree-major within a level.  With
that ordering the two children of parent column c in level l are columns 2c
and 2c+1 of level l+1, so child gather/scatter is pure stride-2 APs.

v3 notes (from the v2 NTFF profile at 321 us):
- All matmul/DVE data is bfloat16 (PSUM accumulation stays fp32): FWL-able
  weights, no fp32r small-moving penalty, 2x DVE on contiguous SBUF ops,
  half the DMA bytes.
- Big levels use three per-gate PSUM tiles on a 3-slot ring (the v2 fused
  [i|o] 4-bank tile serialized the next tile's W-matmuls behind one wide
  2us evacuation, ~2us/tile of PE idle in the top-down pass); small levels
  still fuse all gates into one [128,6C] tile so two activations evacuate
  the whole level.
- The small-level precomputed W-parts are injected with two identity
  matmuls (blocks [i|o] and [u]) instead of six.
- Precompute PSUM chunks ride the same 3-slot "pg" ring, so chunk m+1's
  matmuls overlap chunk m's evacuation (v2 single-buffered these, ~1us
  stall per chunk).
- Top-down weights are DMA'd during the bottom-up small levels.
- The top-down leaf level runs at T=256 (16 tiles) to halve the exposed
  serial gate-chain tail after the last matmul; the per-(half-tree) mean
  partial sums are folded at the end.
- h = sig(o)*tanh(c) at the leaves is fused with the mean accumulation via
  scalar_tensor_tensor (tensor_tensor_reduce's raw-ISA lowering faults
  this runtime).

Exploited zero-fills from the problem spec (verified against the reference
in test.py): h0 == 0, c0 == 0, and all four bias vectors == 0.
"""

import numpy as np

B, NN, XS, H = 64, 1023, 256, 256
NCORES = 8
DEPTH = 9  # levels 0..9, level l has 2^l nodes per tree
TMAX = 512
SM_LEV = 6  # levels 0..SM_LEV-1 get batched W-projections
TLEAF_TD = 256  # top-down leaf tile width (half a tree)

_CACHE = {}

LAST_EXEC_NS = None


def _levels(bl):
    levw = [bl * (1 << l) for l in range(DEPTH + 1)]
    levo = [bl * ((1 << l) - 1) for l in range(DEPTH + 1)]
    tot = bl * NN
    return levw, levo, tot


def _build_nc(bl):
    from concourse import bacc
    import concourse.mybir as mybir
    import concourse.tile as tile

    f32 = mybir.dt.float32
    bf16 = mybir.dt.bfloat16
    Sig = mybir.ActivationFunctionType.Sigmoid
    Tanh = mybir.ActivationFunctionType.Tanh
    MUL = mybir.AluOpType.mult

    LEVW, LEVO, TOT = _levels(bl)
    SM = LEVO[SM_LEV]  # cols of levels 0..SM_LEV-1 (contiguous, level-major)

    nc = bacc.Bacc("TRN2", target_bir_lowering=False)

    xT_d = nc.declare_dram_parameter("xT", [XS, TOT], bf16, isOutput=False)
    w_iou_bu_d = nc.declare_dram_parameter("w_iou_bu_T", [XS, 3 * H], bf16, isOutput=False)
    u_iou_bu_d = nc.declare_dram_parameter("u_iou_bu_T", [H, 3 * H], bf16, isOutput=False)
    u_f_bu_d = nc.declare_dram_parameter("u_f_bu_T", [H, H], bf16, isOutput=False)
    wx_td_d = nc.declare_dram_parameter("wx_iou_td_T", [XS, 3 * H], bf16, isOutput=False)
    wh_td_d = nc.declare_dram_parameter("wh_iou_td_T", [H, 3 * H], bf16, isOutput=False)
    u_iou_td_d = nc.declare_dram_parameter("u_iou_td_T", [H, 3 * H], bf16, isOutput=False)
    u_f_td_d = nc.declare_dram_parameter("u_f_td_T", [H, H], bf16, isOutput=False)
    ident_d = nc.declare_dram_parameter("ident", [128, 128], bf16, isOutput=False)
    out_d = nc.declare_dram_parameter("out", [512, bl], f32, isOutput=True)

    NLEAF_TD = LEVW[DEPTH] // TLEAF_TD  # leaf tiles (2 per tree)

    with tile.TileContext(nc) as tc:
        with (
            tc.tile_pool(name="const", bufs=1) as const,
            tc.tile_pool(name="hbu_pool", bufs=1) as hbu_pool,
            tc.tile_pool(name="work", bufs=2) as work,
            tc.tile_pool(name="xtp", bufs=2) as xtp,
            tc.tile_pool(name="psg", bufs=1, space="PSUM") as psg,
            tc.tile_pool(name="psf", bufs=1, space="PSUM") as psf,
        ):
            # ---- weights (lhsT layout [in_feat, out_feat]) ----
            def load_w(dram, cols, nm):
                ts = []
                for k in (0, 1):
                    tag, nb = ("w768", 10) if cols == 768 else ("uf", 4)
                    t = const.tile([128, cols], bf16, name=f"{nm}{k}", tag=tag, bufs=nb)
                    # weight loads go on the Scalar HWDGE queue so they don't
                    # serialize behind the xt streaming loads on Sync
                    nc.scalar.dma_start(out=t, in_=dram[k * 128:(k + 1) * 128, :])
                    ts.append(t)
                return ts

            w_bu = load_w(w_iou_bu_d, 3 * H, "wbu")
            ident = const.tile([128, 128], bf16, name="ident", tag="ident")
            nc.scalar.dma_start(out=ident, in_=ident_d[:, :])
            u_bu = uf_bu = None  # loaded lazily once the leaf level is emitted
            wx_td = wh_td = u_td = uf_td = None

            hbu = hbu_pool.tile([128, 2 * TOT], bf16, name="hbu", tag="hbu")
            # leaf-mean partials per (g, leaf tile); folded to [128,2,bl] at end
            mean = const.tile([128, 2 * NLEAF_TD], f32, name="mean", tag="mean")
            rootf = const.tile([128, 2, bl], f32, name="rootf", tag="rootf")
            meanf = const.tile([128, 2, bl], f32, name="meanf", tag="meanf")

            # X^T for the small levels, kept for both precompute passes
            xsm = const.tile([128, 2 * SM], bf16, name="xsm", tag="xsm")

            def load_xsm():
                for k in (0, 1):
                    nc.scalar.dma_start(
                        out=xsm[:, k * SM:(k + 1) * SM],
                        in_=xT_d[k * 128:(k + 1) * 128, 0:SM],
                    )

            def load_x(off, o0, T):
                xt = xtp.tile([128, 2 * T], bf16, name="xt", tag="xt", bufs=3)
                for k in (0, 1):
                    nc.sync.dma_start(
                        out=xt[:, k * T:(k + 1) * T],
                        in_=xT_d[k * 128:(k + 1) * 128, off + o0:off + o0 + T],
                    )
                return xt

            def g2(ap, width):
                return ap.rearrange("p (g c) -> p g c", g=2)

            # ---- gate psum helpers ----
            # Big levels: three [128, 2T] tiles (i, o, u) on one 3-slot ring;
            # the weight row slice for (gate gi, half g) is (2*gi+g)*128.
            # `pre` injects the precomputed W-part with one identity matmul
            # per gate (rhs pre[:, 2gi:2gi+2, ...], both g halves at once).
            # Tiny levels (6C <= 1024): one fused [128, 6C] tile on the same
            # ring, block order [i_g0, i_g1, o_g0, o_g1, u_g0, u_g1].
            def iou_mms_big(T, phase1, phase2=None, pre=None, lev=0):
                merged = phase2 is not None and T < 512
                p1 = phase1 + phase2 if merged else phase1
                pending = phase2 is not None and not merged
                lo = LEVO[lev]
                pg = {}
                for gi, gate in enumerate(("i", "o", "u")):
                    p = psg.tile([128, 2 * T], f32, name=f"pg{gate}", tag="pg", bufs=3)
                    pg[gate] = p
                    if pre is not None:
                        # one identity matmul covers both g halves; with pre
                        # the tile is a single PSUM bank (T <= 256), so stop
                        # may only be set on the very last matmul of the tile
                        nc.tensor.matmul(
                            p[:, 0:2 * T], ident, pre[:, 2 * gi:2 * gi + 2, lo:lo + T],
                            start=True, stop=(not p1 and not pending),
                        )
                    for g in (0, 1):
                        ms = slice((2 * gi + g) * 128, (2 * gi + g + 1) * 128)
                        mms = [
                            (pair[k][:, ms], rhs(k))
                            for pair, rhs in p1
                            for k in (0, 1)
                        ]
                        for i, (lhs, rhs) in enumerate(mms):
                            last = i == len(mms) - 1 and (pre is None or g == 1)
                            nc.tensor.matmul(
                                p[:, g * T:(g + 1) * T],
                                lhs,
                                rhs,
                                start=(i == 0 and pre is None),
                                stop=(not pending and last),
                            )

                def close():
                    if not pending:
                        return
                    for gi2, gate in enumerate(("i", "o", "u")):
                        p = pg[gate]
                        for g in (0, 1):
                            ms = slice((2 * gi2 + g) * 128, (2 * gi2 + g + 1) * 128)
                            mms = [
                                (pair[k][:, ms], rhs(k))
                                for pair, rhs in phase2
                                for k in (0, 1)
                            ]
                            for i, (lhs, rhs) in enumerate(mms):
                                nc.tensor.matmul(
                                    p[:, g * T:(g + 1) * T],
                                    lhs,
                                    rhs,
                                    start=False,
                                    stop=(i == len(mms) - 1),
                                )

                return ("big", pg), close

            def iou_mms_small(T, phase1, pre, lev):
                """One fused [128, 6T] psum (6T <= 1024, rides the pg ring).
                Identity matmuls inject pre per bank-group; since PSUM
                start/stop state is per 2KB bank, stop is set only on the
                last matmul touching each bank."""
                p6 = psg.tile([128, 6 * T], f32, name="p6", tag="pg", bufs=3)
                lo = LEVO[lev]
                # bank groups: one if the whole tile fits a 2KB bank
                groups = [(0, 6)] if 6 * T * 4 <= 2048 else [(0, 4), (4, 6)]
                last_blk = {}
                for b0, b1 in groups:
                    nc.tensor.matmul(
                        p6[:, b0 * T:b1 * T], ident, pre[:, b0:b1, lo:lo + T],
                        start=True, stop=(not phase1),
                    )
                    last_blk[b1 - 1] = True
                for gi in range(3):
                    for g in (0, 1):
                        ms = slice((2 * gi + g) * 128, (2 * gi + g + 1) * 128)
                        blk = 2 * gi + g
                        mms = [
                            (pair[k][:, ms], rhs(k))
                            for pair, rhs in phase1
                            for k in (0, 1)
                        ]
                        for i, (lhs, rhs) in enumerate(mms):
                            nc.tensor.matmul(
                                p6[:, blk * T:(blk + 1) * T],
                                lhs,
                                rhs,
                                start=False,
                                stop=(i == len(mms) - 1 and blk in last_blk),
                            )
                return ("small", p6), lambda: None

            def precompute(rhs_for):
                """pre[:, m, :] = sum over (lhsT_pair, rhs_slicer): lhsT.T @ rhs
                over the SM small-level columns.  Rides the pg ring so chunk
                m+1's matmuls overlap chunk m's evacuation."""
                pre = const.tile([128, 6, SM], bf16, name="pre", tag="pre")
                for m in range(6):
                    p = psg.tile([128, SM], f32, name="pp", tag="pg", bufs=3)
                    mms = []
                    for pair, rhs in rhs_for:
                        for k in (0, 1):
                            mms.append((pair[k][:, m * 128:(m + 1) * 128], rhs(k)))
                    for i, (lhs, rhs) in enumerate(mms):
                        nc.tensor.matmul(
                            p, lhs, rhs, start=(i == 0), stop=(i == len(mms) - 1)
                        )
                    nc.vector.tensor_copy(pre[:, m, :], p)
                return pre

            def gates(pg, T, c_red, c_out, h_out, leaf_tile=None):
                """pg: ("big", {i,o,u}) or ("small", fused tile).
                c_red: None | ("full", ap[128,2,T]) | ("parent", ap[128,2,pT])
                c_out/h_out: [128, 2, T] views; leaf_tile: tile index for the
                fused leaf-mean path (h_out unused)."""
                kind, p = pg
                if kind == "big":
                    si = work.tile([128, 2 * T], bf16, name="si", tag="ga")
                    nc.scalar.activation(si, p["i"], Sig)
                    so = work.tile([128, 2 * T], bf16, name="so", tag="gb", bufs=3)
                    nc.scalar.activation(so, p["o"], Sig)
                    tu = work.tile([128, 2 * T], bf16, name="tu", tag="gb", bufs=3)
                    nc.scalar.activation(tu, p["u"], Tanh)
                else:
                    sio = work.tile([128, 4 * T], bf16, name="sio", tag="ga")
                    nc.scalar.activation(sio, p[:, 0:4 * T], Sig)
                    tu = work.tile([128, 2 * T], bf16, name="tu", tag="gb", bufs=3)
                    nc.scalar.activation(tu, p[:, 4 * T:6 * T], Tanh)
                    si = sio[:, 0:2 * T]
                    so = sio[:, 2 * T:4 * T]
                if c_red is None:
                    nc.vector.tensor_mul(c_out, g2(si, T), g2(tu, T))
                else:
                    nc.vector.tensor_mul(si, si, tu)  # situ, in place
                    knd, cr = c_red
                    if knd == "full":
                        nc.vector.tensor_add(c_out, g2(si, T), cr)
                    else:  # parent-granularity c_red, broadcast to child pairs
                        pT = T // 2
                        si4 = si.rearrange("p (g n two) -> p g n two", g=2, two=2)
                        co4 = c_out.rearrange("p g (n two) -> p g n two", two=2)
                        crb = cr.to_broadcast([128, 2, pT, 2])
                        nc.vector.tensor_add(co4, si4, crb)
                tct = work.tile([128, 2 * T], bf16, name="tct", tag="gc", bufs=2)
                nc.scalar.activation(g2(tct, T), c_out, Tanh)
                if leaf_tile is None:
                    nc.vector.tensor_mul(h_out, g2(so, T), g2(tct, T))
                else:
                    # fused h = sig(o)*tanh(c) + per-(g,tile) mean partials
                    scr = work.tile([128, 2 * T], bf16, name="scr", tag="fc", bufs=2)
                    for g in (0, 1):
                        nc.vector.scalar_tensor_tensor(
                            scr[:, g * T:(g + 1) * T],
                            so[:, g * T:(g + 1) * T],
                            1.0 / (1 << DEPTH),
                            tct[:, g * T:(g + 1) * T],
                            MUL,
                            MUL,
                            accum_out=mean[:, g * NLEAF_TD + leaf_tile:
                                           g * NLEAF_TD + leaf_tile + 1],
                        )

            # ================= bottom-up =================
            pre_bu = None
            with tc.tile_pool(name="bu_state", bufs=1) as bu_state:
                c_next = None
                C_next = 0
                for l in range(DEPTH, -1, -1):
                    if l == SM_LEV - 1 and pre_bu is None:
                        load_xsm()
                        pre_bu = precompute(
                            [(w_bu, lambda k: xsm[:, k * SM:(k + 1) * SM])]
                        )
                        # td weights: DMA during the bu small levels (the
                        # scalar queue is idle here; pre_td needs them soon)
                        wx_td = load_w(wx_td_d, 3 * H, "wxtd")
                        wh_td = load_w(wh_td_d, 3 * H, "whtd")
                        u_td = load_w(u_iou_td_d, 3 * H, "utd")
                        uf_td = load_w(u_f_td_d, H, "uftd")
                    if l == DEPTH - 1 and u_bu is None:
                        u_bu = load_w(u_iou_bu_d, 3 * H, "ubu")
                        uf_bu = load_w(u_f_bu_d, H, "ufbu")
                    C, off = LEVW[l], LEVO[l]
                    T = min(TMAX, C)
                    leaf = l == DEPTH
                    small = l < SM_LEV
                    par = "A" if l % 2 else "Bp"
                    c_cur = bu_state.tile(
                        [128, 2 * C], bf16, name=f"c{l}", tag=f"c{par}"
                    )
                    choff = LEVO[l + 1] if not leaf else 0
                    ntile = C // T
                    # hsum for the whole level up front: it only needs the
                    # previous level's h, and putting it first in the DVE
                    # queue keeps the iou U-matmuls from waiting behind the
                    # previous tile's situ/c/h chain
                    hsums = []
                    if not leaf:
                        for j in range(ntile):
                            o0 = j * T
                            ncj = 2 if 2 * T > TMAX else 1
                            Tc = 2 * T // ncj
                            hsum = work.tile(
                                [128, 2 * T], bf16, name="hsum", tag="hsum", bufs=3
                            )
                            for cj in range(ncj):
                                cb = choff + 2 * o0 + cj * Tc
                                h2 = Tc // 2
                                hsv = g2(hsum, T)[:, :, cj * h2:(cj + 1) * h2]
                                hb4 = hbu.rearrange("p (k c) -> p k c", k=2)[
                                    :, :, cb:cb + Tc
                                ].rearrange("p k (n two) -> p k n two", two=2)
                                nc.vector.tensor_add(
                                    hsv, hb4[:, :, :, 0], hb4[:, :, :, 1]
                                )
                            hsums.append(hsum)
                    for j in range(ntile):
                        o0 = j * T
                        xt = None if small else load_x(off, o0, T)
                        cred = None
                        hsum = None
                        pg = close = None
                        u_phase = None
                        if not leaf:
                            ncj = 2 if 2 * T > TMAX else 1
                            Tc = 2 * T // ncj
                            cred = work.tile(
                                [128, 2 * T], bf16, name="cred", tag="cred"
                            )
                            hsum = hsums[j]
                            hs_ = hsum
                            u_phase = [
                                (u_bu, lambda k, h=hs_: h[:, k * T:(k + 1) * T])
                            ]
                        if not small:
                            # W-matmuls up front: they only need xt, so PE has
                            # work while the f chains of this tile run
                            xt_ = xt
                            pg, close = iou_mms_big(
                                T,
                                [(w_bu, lambda k, x=xt_: x[:, k * T:(k + 1) * T])],
                                u_phase,
                            )
                        if not leaf:
                            for cj in range(ncj):
                                cb = choff + 2 * o0 + cj * Tc
                                pf = psf.tile(
                                    [128, 2 * Tc], f32, name="pf", tag="pf"
                                )
                                for g in (0, 1):
                                    for k in (0, 1):
                                        nc.tensor.matmul(
                                            pf[:, g * Tc:(g + 1) * Tc],
                                            uf_bu[k][:, g * 128:(g + 1) * 128],
                                            hbu[:, k * TOT + cb:k * TOT + cb + Tc],
                                            start=(k == 0),
                                            stop=(k == 1),
                                        )
                                fsb = work.tile(
                                    [128, 2 * Tc], bf16, name="fsb", tag="fsb"
                                )
                                nc.scalar.activation(fsb, pf, Sig)
                                # fc = f * c_child (bf16 SBUF-only, DVE 2x)
                                cv = g2(c_next, C_next)[
                                    :, :, 2 * o0 + cj * Tc:2 * o0 + (cj + 1) * Tc
                                ]
                                fct = work.tile(
                                    [128, 2 * Tc], bf16, name="fct", tag="fc"
                                )
                                nc.vector.tensor_mul(g2(fct, Tc), g2(fsb, Tc), cv)
                                # c_red halves: pairwise sums of fc
                                h2 = Tc // 2
                                crv = g2(cred, T)[:, :, cj * h2:(cj + 1) * h2]
                                fc4 = fct.rearrange(
                                    "p (g n two) -> p g n two", g=2, two=2
                                )
                                eng = nc.vector if small else nc.gpsimd
                                eng.tensor_add(
                                    crv, fc4[:, :, :, 0], fc4[:, :, :, 1]
                                )
                        if small:
                            if 6 * T <= 1024:
                                pg, close = iou_mms_small(T, u_phase, pre_bu, l)
                            else:
                                pg, close = iou_mms_big(
                                    T, u_phase, pre=pre_bu, lev=l
                                )
                        else:
                            close()
                        cr = None if leaf else ("full", g2(cred, T))
                        gates(
                            pg,
                            T,
                            cr,
                            g2(c_cur, C)[:, :, o0:o0 + T],
                            hbu.rearrange("p (k c) -> p k c", k=2)[
                                :, :, off + o0:off + o0 + T
                            ],
                        )
                    c_next = c_cur
                    C_next = C

            # root h (f32 copy for output; DMA cannot convert dtypes)
            nc.vector.tensor_copy(
                rootf, hbu.rearrange("p (k c) -> p k c", k=2)[:, :, 0:bl]
            )

            # ---- td precompute: Wx@x + Wh@h_bu over small-level cols ----
            pre_td = precompute(
                [
                    (wx_td, lambda k: xsm[:, k * SM:(k + 1) * SM]),
                    (wh_td, lambda k: hbu[:, k * TOT:k * TOT + SM]),
                ],
            )

            # ================= top-down =================
            with tc.tile_pool(name="td_state", bufs=1) as td_state:
                h_prev = c_prev = None
                C_prev = 0
                for l in range(0, DEPTH + 1):
                    C, off = LEVW[l], LEVO[l]
                    leaf = l == DEPTH
                    root = l == 0
                    small = l < SM_LEV
                    T = TLEAF_TD if leaf else min(TMAX, C)
                    par = "A" if l % 2 else "Bp"
                    if not leaf:
                        h_cur = td_state.tile(
                            [128, 2 * C], bf16, name=f"th{l}", tag=f"th{par}"
                        )
                        c_cur = td_state.tile(
                            [128, 2 * C], bf16, name=f"tc{l}", tag=f"tc{par}"
                        )
                    else:
                        h_cur = c_cur = None
                    for j in range(C // T):
                        o0 = j * T
                        xt = None if small else load_x(off, o0, T)
                        credp = None
                        pT = T // 2 if not root else 0
                        po = o0 // 2
                        u_phase = None
                        if not root:
                            hp_, po_, pT_, Cp_ = h_prev, po, pT, C_prev
                            u_phase = [
                                (
                                    u_td,
                                    lambda k, h=hp_, a=po_, b=pT_, Cp=Cp_: h[
                                        :, k * Cp + a:k * Cp + a + b
                                    ].to_broadcast([128, b, 2]),
                                )
                            ]
                        if not small:
                            # Wx/Wh matmuls first (inputs all ready)
                            xt_ = xt
                            pg, close = iou_mms_big(
                                T,
                                [
                                    (wx_td, lambda k, x=xt_: x[:, k * T:(k + 1) * T]),
                                    (
                                        wh_td,
                                        lambda k, a=off + o0: hbu[
                                            :, k * TOT + a:k * TOT + a + T
                                        ],
                                    ),
                                ],
                                u_phase,
                            )
                        if not root:
                            pf = psf.tile([128, 2 * pT], f32, name="pftd", tag="pf")
                            for g in (0, 1):
                                for k in (0, 1):
                                    nc.tensor.matmul(
                                        pf[:, g * pT:(g + 1) * pT],
                                        uf_td[k][:, g * 128:(g + 1) * 128],
                                        h_prev[:, k * C_prev + po:k * C_prev + po + pT],
                                        start=(k == 0),
                                        stop=(k == 1),
                                    )
                            fsb = work.tile(
                                [128, 2 * pT], bf16, name="fsbtd", tag="fsb"
                            )
                            nc.scalar.activation(fsb, pf, Sig)
                            credp = work.tile(
                                [128, 2 * pT], bf16, name="credp", tag="cred"
                            )
                            nc.vector.tensor_mul(
                                g2(credp, pT),
                                g2(fsb, pT),
                                g2(c_prev, C_prev)[:, :, po:po + pT],
                            )
                        if small:
                            if 6 * T <= 1024:
                                pg, close = iou_mms_small(
                                    T, u_phase or [], pre_td, l
                                )
                            else:
                                pg, close = iou_mms_big(
                                    T, u_phase or [], pre=pre_td, lev=l
                                )
                        else:
                            close()
                        cr = None if root else ("parent", g2(credp, pT))
                        if leaf:
                            gates(pg, T, cr, g2(
                                work.tile([128, 2 * T], bf16, name="cl", tag="cl"),
                                T,
                            ), None, leaf_tile=j)
                        else:
                            gates(
                                pg,
                                T,
                                cr,
                                g2(c_cur, C)[:, :, o0:o0 + T],
                                g2(h_cur, C)[:, :, o0:o0 + T],
                            )
                    h_prev, c_prev = h_cur, c_cur
                    C_prev = C

            # ---- outputs ----
            # fold leaf-tile mean partials (2 tiles per tree) into [128,2,bl]
            mp = mean.rearrange("p (g t two) -> p g t two", g=2, two=2)
            nc.vector.tensor_add(meanf, mp[:, :, :, 0], mp[:, :, :, 1])
            nc.sync.dma_start(
                out=out_d[0:256, :].rearrange("(k p) b -> p k b", k=2),
                in_=rootf,
            )
            nc.sync.dma_start(
                out=out_d[256:512, :].rearrange("(g p) b -> p g b", g=2),
                in_=meanf,
            )

    if not nc.is_finalized():
        nc.finalize()
    return nc


def _to_bf16(a):
    import ml_dtypes

    return np.ascontiguousarray(np.asarray(a, np.float32)).astype(ml_dtypes.bfloat16)


def _prep_shared(inputs):
    """Weight marshaling shared by all cores (biases are zero by spec)."""
    W_iou_td = np.asarray(inputs["W_iou_td"], np.float32)
    return {
        "w_iou_bu_T": _to_bf16(np.asarray(inputs["W_iou_bu"], np.float32).T),
        "u_iou_bu_T": _to_bf16(np.asarray(inputs["U_iou_bu"], np.float32).T),
        "u_f_bu_T": _to_bf16(np.asarray(inputs["U_f_bu"], np.float32).T),
        "wx_iou_td_T": _to_bf16(W_iou_td[:, :XS].T),
        "wh_iou_td_T": _to_bf16(W_iou_td[:, XS:].T),
        "u_iou_td_T": _to_bf16(np.asarray(inputs["U_iou_td"], np.float32).T),
        "u_f_td_T": _to_bf16(np.asarray(inputs["U_f_td"], np.float32).T),
        "ident": _to_bf16(np.eye(128, dtype=np.float32)),
    }


def prep_xt(Xc):
    """[bl, NN, XS] -> [XS, bl*NN] bf16 with level-major column blocks."""
    bl = Xc.shape[0]
    xt = np.asarray(Xc, np.float32).transpose(2, 0, 1)  # [XS, bl, NN]
    blocks = []
    for l in range(DEPTH + 1):
        lo, nl = (1 << l) - 1, 1 << l
        blocks.append(xt[:, :, lo:lo + nl].reshape(XS, bl * nl))
    return _to_bf16(np.concatenate(blocks, axis=1))


def unpack_out(o, bl):
    """[512, bl] -> [bl, 512] (root_h_bu | leaf mean)."""
    return np.concatenate([o[0:256, :].T, o[256:512, :].T], axis=1)


def kernel(**inputs):
    global LAST_EXEC_NS
    from concourse.bass_utils import run_bass_kernel_spmd

    bl = B // NCORES
    if "nc" not in _CACHE:
        _CACHE["nc"] = _build_nc(bl)
    nc = _CACHE["nc"]

    shared = _prep_shared(inputs)
    X = np.asarray(inputs["X"], np.float32)
    in_maps = []
    for c in range(NCORES):
        m = dict(shared)
        m["xT"] = prep_xt(X[c * bl:(c + 1) * bl])
        in_maps.append(m)

    trace = _CACHE.get("trace", False)
    res = None
    for attempt in range(3):
        try:
            res = run_bass_kernel_spmd(nc, in_maps, list(range(NCORES)), trace=trace)
            break
        except Exception:
            # transient NRT device faults have been observed; retry
            if attempt == 2:
                raise
            import time

            time.sleep(5)
    LAST_EXEC_NS = res.exec_time_ns
    _CACHE["last_results"] = res

    out = np.concatenate(
        [unpack_out(res.results[c]["out"], bl) for c in range(NCORES)], axis=0
    )
    return out.astype(np.float32)
